# revision 29
# baseline (speedup 1.0000x reference)
"""MMoE-style CustomizedGateControl kernel for 8x TRN2 NeuronCores.

Data-parallel over the batch dim (16384 -> 8 x 2048). Per core:
  - expert GEMMs + per-task gates as one wide f16 matmul sweep with batch
    rows on PSUM partitions, emitted in i-quarter phases interleaved with
    the gated-combine + tower work so the PE never starves
  - epilogue: DVE adds bias in-place in PSUM, ACT applies ReLU on the
    PSUM->SBUF f16 drain (cheaper PSUM read path for ACT)
  - gated combine fused with the [b,e]->[e,b] transpose as f16 PE matmuls:
    info_t.T = sum_g X_g.T @ diag(gate_tg); diag built on GPSIMD
  - tower MLP GEMMs in f16, interleaved per 512-batch chunk
All parameters replicated; no collectives.
"""

import sys

if "/opt/trn_rl_repo" not in sys.path:
    sys.path.insert(0, "/opt/trn_rl_repo")

import numpy as np

import concourse.bacc as bacc
import concourse.mybir as mybir
import concourse.tile as tile
from concourse.bass_utils import run_bass_kernel_spmd

# problem dims
B, D, E, H = 16384, 512, 256, 128
S, K, T = 4, 4, 2
NCORES = 8
BC = B // NCORES          # 2048 batch rows per core
P = 128                   # partitions
NB = BC // P              # 16 b-tiles per core
NE = S + T * K            # 12 experts
G = S + K                 # 8 gate inputs per task
WCOLS = NE * E            # 3072 expert output columns
WALL = WCOLS + T * G      # 3088 = experts + gate columns

f32 = mybir.dt.float32
f16 = mybir.dt.float16

# expert-sweep column groups in expert-column space (first ones small so PE
# can start before the bulk of the weights arrive); wall stores the gate
# columns FIRST, so wall col = expert col + T*G and group 0's matmul also
# produces the gates
GROUPS = [(512, 1024), (0, 128), (1024, 1536), (1536, 2048),
          (2048, 2560), (2560, 3072), (128, 256), (256, 512)]
NPHASE = 4                # i-quarter phases of 4 b-tiles each


def _expert_col(t: int, j: int) -> int:
    """Column offset in the fused expert output for gate input j of task t."""
    if j < S:
        return j * E                      # shared expert j
    return (S + t * K + (j - S)) * E      # task expert (t, j-S)


def _build():
    nc = bacc.Bacc("TRN2", target_bir_lowering=False, debug=False)

    xt_d = nc.dram_tensor("xt", [D, BC], f16, kind="ExternalInput").ap()
    wall_d = nc.dram_tensor("wall", [D, WALL], f16, kind="ExternalInput").ap()
    biasb_d = nc.dram_tensor("biasb", [P, WCOLS], f16, kind="ExternalInput").ap()
    tw1_d = nc.dram_tensor("tw1", [T, E, H], f16, kind="ExternalInput").ap()
    tb1_d = nc.dram_tensor("tb1", [H, T], f32, kind="ExternalInput").ap()
    tw2_d = nc.dram_tensor("tw2", [H, T], f16, kind="ExternalInput").ap()
    ident_d = nc.dram_tensor("ident", [P, P], f16, kind="ExternalInput").ap()
    out_d = nc.dram_tensor("out", [T, BC], f32, kind="ExternalOutput").ap()

    KC = D // P  # 4 contraction chunks

    with tile.TileContext(nc) as tc:
        with (
            tc.tile_pool(name="const", bufs=1) as const,
            tc.tile_pool(name="dg", bufs=8) as dg_pool,
            tc.tile_pool(name="hsb", bufs=2) as hsb_pool,
            tc.tile_pool(name="osb", bufs=2) as osb_pool,
        ):
            xt_t = [const.tile([P, BC], f16, tag=f"xt{k}", name=f"xt{k}") for k in range(KC)]
            wall_t = [const.tile([P, WALL], f16, tag=f"wall{k}", name=f"wall{k}") for k in range(KC)]
            biasb = const.tile([P, WCOLS], f16, tag="biasb", name="biasb")
            ident = const.tile([P, P], f16, tag="ident", name="ident")

            # ---- staged input DMA, spread across four issue queues, in the
            # order the expert sweep consumes it (wall group cols + xt b-range)
            # ACT (scalar) is needed for the epilogue within ~12us of start —
            # issuing DMAs there blocks its sequencer on the DGE ring, so the
            # critical loads alternate gpsimd/sync only
            engs = [nc.gpsimd, nc.sync]
            ei = 0

            def dma(dst, src):
                nonlocal ei
                engs[ei % 2].dma_start(dst, src)
                ei += 1

            # strictly in phase-0 consumption order: wall group cols (+bias
            # half-a-group behind), xt per i-quarter just in time
            for k in range(KC):
                rs = slice(k * P, (k + 1) * P)
                dma(wall_t[k][:, 528:1040], wall_d[rs, 528:1040])
            dma(biasb[:, 512:1024], biasb_d[:, 512:1024])
            for k in range(KC):
                rs = slice(k * P, (k + 1) * P)
                dma(xt_t[k][:, 0:P], xt_d[rs, 0:P])
            for k in range(KC):
                rs = slice(k * P, (k + 1) * P)
                dma(wall_t[k][:, 0:144], wall_d[rs, 0:144])
            dma(biasb[:, 0:512], biasb_d[:, 0:512])
            for k in range(KC):
                rs = slice(k * P, (k + 1) * P)
                dma(xt_t[k][:, P:256], xt_d[rs, P:256])
            for k in range(KC):
                rs = slice(k * P, (k + 1) * P)
                dma(xt_t[k][:, 256:512], xt_d[rs, 256:512])
            for c0 in range(1040, WALL, 512):
                c1 = min(c0 + 512, WALL)
                for k in range(KC):
                    rs = slice(k * P, (k + 1) * P)
                    dma(wall_t[k][:, c0:c1], wall_d[rs, c0:c1])
                dma(biasb[:, c0 - T * G:c1 - T * G], biasb_d[:, c0 - T * G:c1 - T * G])
            for k in range(KC):
                rs = slice(k * P, (k + 1) * P)
                nc.scalar.dma_start(xt_t[k][:, 512:1024], xt_d[rs, 512:1024])
            for k in range(KC):
                rs = slice(k * P, (k + 1) * P)
                dma(wall_t[k][:, 144:272], wall_d[rs, 144:272])
            for k in range(KC):
                rs = slice(k * P, (k + 1) * P)
                dma(wall_t[k][:, 272:528], wall_d[rs, 272:528])
            nc.scalar.dma_start(ident[:], ident_d[:])

            tw1_t = {}
            for t in range(T):
                for kc in range(2):
                    t_ = const.tile([P, H], f16, tag=f"tw1_{t}_{kc}", name=f"tw1_{t}_{kc}")
                    dma(t_[:], tw1_d[t, kc * P:(kc + 1) * P, :])
                    tw1_t[(t, kc)] = t_
            tb1 = const.tile([H, T], f32, tag="tb1", name="tb1")
            dma(tb1[:], tb1_d[:])
            tw2 = const.tile([H, T], f16, tag="tw2", name="tw2")
            dma(tw2[:], tw2_d[:])
            # late-needed xt quarters on the scalar queue (issued up front in
            # its FIFO but few enough not to delay the first relu)
            for k in range(KC):
                rs = slice(k * P, (k + 1) * P)
                nc.scalar.dma_start(xt_t[k][:, 1024:2048], xt_d[rs, 1024:2048])

            exp_sb_t = [
                const.tile([P, WCOLS], f16, tag=f"expsb{i}", name=f"expsb{i}")
                for i in range(NB)
            ]
            gsb_t = [
                const.tile([P, T * G], f16, tag=f"gsb{i}", name=f"gsb{i}")
                for i in range(NB)
            ]
            # info_t.T, per e-chunk; columns are t-major: [t0 | t1] each BC wide
            infoT = [
                const.tile([P, T * BC], f16, tag=f"infoT{ec}", name=f"infoT{ec}")
                for ec in range(2)
            ]

            with (
                tc.tile_pool(name="expps", bufs=4, space="PSUM") as expps_pool,
                tc.tile_pool(name="ctps", bufs=2, space="PSUM") as ctps_pool,
                tc.tile_pool(name="hps", bufs=2, space="PSUM") as hps_pool,
            ):
                # ~10 dummy matmuls on garbage SBUF while the input DMA is
                # still in flight: keeps the PE busy through the HAM SHORT
                # window so the real sweep starts already at full clock
                warm_ps = expps_pool.tile([P, 512], f32, tag="ps", name="warm")
                for w in range(22):
                    c = (w % 5) * 512
                    nc.tensor.matmul(
                        warm_ps[:],
                        exp_sb_t[14][:, 0:P],
                        exp_sb_t[15][:, c:c + 512],
                        start=True,
                        stop=True,
                    )

                def expert_phase(p):
                    i0, i1 = 4 * p, 4 * p + 4
                    TG = T * G
                    for c0, c1 in GROUPS:
                        # the c0==0 group's matmul covers the gate columns too
                        has_g = c0 == 0
                        w0 = 0 if has_g else c0 + TG
                        w1 = c1 + TG
                        cw = w1 - w0
                        po = TG if has_g else 0  # expert data offset in psum
                        for i in range(i0, i1):
                            bs = slice(i * P, (i + 1) * P)
                            ps = expps_pool.tile([P, 512], f32, tag="ps", name="ps")
                            for k in range(KC):
                                nc.tensor.matmul(
                                    ps[:, 0:cw],
                                    xt_t[k][:, bs],
                                    wall_t[k][:, w0:w1],
                                    start=(k == 0),
                                    stop=(k == KC - 1),
                                )
                            nc.vector.tensor_add(
                                ps[:, po:cw], ps[:, po:cw], biasb[:, c0:c1]
                            )
                            if (c0, c1) == GROUPS[-1]:
                                # last group per tile: keep the whole drain on
                                # DVE so the ACT FIFO can't delay combine(i)
                                nc.vector.tensor_scalar_max(
                                    exp_sb_t[i][:, c0:c1], ps[:, po:cw], 0.0
                                )
                            else:
                                nc.scalar.activation(
                                    exp_sb_t[i][:, c0:c1],
                                    ps[:, po:cw],
                                    mybir.ActivationFunctionType.Relu,
                                )
                            if has_g:
                                nc.scalar.copy(gsb_t[i][:], ps[:, 0:TG])
                                # diag built on GPSIMD; dg pool depth throttles
                                diag = dg_pool.tile([P, TG * P], f16, tag="dg", name=f"dg{i}")
                                nc.gpsimd.tensor_mul(
                                    diag[:].rearrange("p (j c) -> p j c", c=P),
                                    ident[:, None, :].broadcast_to([P, TG, P]),
                                    gsb_t[i][:, :, None].broadcast_to([P, TG, P]),
                                )
                                diag_t[i] = diag

                def combine(i, drain_dve=False):
                    bs = slice(i * P, (i + 1) * P)
                    exp_sb = exp_sb_t[i]
                    diag = diag_t[i]
                    for ec in range(2):
                        ct = ctps_pool.tile([P, T * P], f32, tag="ctps", name="ctps")
                        for g in range(S):
                            c = _expert_col(0, g)
                            nc.tensor.matmul(
                                ct[:],
                                exp_sb[:, c + ec * P: c + (ec + 1) * P],
                                diag[:, g * 2 * P:(g * 2 + 2) * P],
                                start=(g == 0),
                                stop=False,
                                skip_group_check=True,
                            )
                        for t in range(T):
                            for g in range(S, G):
                                c = _expert_col(t, g)
                                nc.tensor.matmul(
                                    ct[:, t * P:(t + 1) * P],
                                    exp_sb[:, c + ec * P: c + (ec + 1) * P],
                                    diag[:, (g * 2 + t) * P:(g * 2 + t + 1) * P],
                                    start=False,
                                    stop=(g == G - 1),
                                    skip_group_check=True,
                                )
                        # drain both tasks in one copy into t-major infoT
                        # (DVE for the tail quarter, where ACT is the pacer)
                        dst = infoT[ec][:].rearrange("p (t n) -> p t n", t=T)[:, :, bs]
                        srcv = ct[:].rearrange("p (t c) -> p t c", t=T)
                        if drain_dve:
                            nc.vector.tensor_copy(dst, srcv)
                        else:
                            nc.scalar.copy(dst, srcv)

                def towers(c, b0, b1, hs_dve=False):
                    # tower MLP for batch cols [b0:b1) of 512-chunk c
                    w = b1 - b0
                    for t in range(T):
                        hp = hps_pool.tile([P, 512], f32, tag="hps", name="hps")
                        for kc in range(2):
                            nc.tensor.matmul(
                                hp[:, 0:w],
                                tw1_t[(t, kc)][:],
                                infoT[kc][:, t * BC + b0: t * BC + b1],
                                start=(kc == 0),
                                stop=(kc == 1),
                            )
                        hs = hsb_pool.tile([P, 512], f16, tag="hsb", name="hsb")
                        if hs_dve and t == 0:
                            nc.vector.tensor_scalar(
                                hs[:, 0:w],
                                hp[:, 0:w],
                                tb1[:, t:t + 1],
                                0.0,
                                mybir.AluOpType.add,
                                mybir.AluOpType.max,
                            )
                        else:
                            nc.scalar.activation(
                                hs[:, 0:w],
                                hp[:, 0:w],
                                mybir.ActivationFunctionType.Relu,
                                bias=tb1[:, t:t + 1],
                            )
                        op = hps_pool.tile([P, 512], f32, tag="hps", name="hps")
                        nc.tensor.matmul(
                            op[0:1, 0:w],
                            tw2[:, t:t + 1],
                            hs[:, 0:w],
                            start=True,
                            stop=True,
                        )
                        ob = osb_pool.tile([1, 512], f32, tag="osb", name="osb")
                        nc.scalar.copy(ob[0:1, 0:w], op[0:1, 0:w])
                        nc.sync.dma_start(
                            out_d.rearrange("t n -> (t n)")[
                                None, (t * BC + b0):(t * BC + b1)
                            ],
                            ob[0:1, 0:w],
                        )

                diag_t = {}
                # phase-interleaved emission: experts for quarter p, then
                # combines+towers for quarter p-1 (so PE always has both
                # streams ready and diag/epilogue engines stay ahead)
                expert_phase(0)
                expert_phase(1)
                for i in range(0, 4):
                    combine(i)
                towers(0, 0, 512)
                expert_phase(2)
                for i in range(4, 8):
                    combine(i)
                towers(1, 512, 1024)
                expert_phase(3)
                for i in range(8, 12):
                    combine(i)
                towers(2, 1024, 1536)
                combine(12, drain_dve=True)
                combine(13, drain_dve=True)
                # progressively finer tail tower pieces so the dependency
                # chain after the last combine is short; tail drains go to
                # DVE (idle by then) so ACT's FIFO isn't the pacer
                towers(3, 1536, 1792, hs_dve=True)
                combine(14, drain_dve=True)
                towers(3, 1792, 1920, hs_dve=True)
                combine(15, drain_dve=True)
                towers(3, 1920, 2048, hs_dve=True)

    nc.compile()
    return nc


_NC = None


def _get_nc():
    global _NC
    if _NC is None:
        _NC = _build()
    return _NC


def _prep_shared(shared_W, shared_b, task_W, task_b, gate_W, tower_W1, tower_b1, tower_W2):
    gwi = np.empty((D, T * G), np.float32)
    for t in range(T):
        gwi[:, t::T] = np.asarray(gate_W[t])  # column g*T+t = gate (t, g)
    cols = [gwi]  # gate columns first
    cols += [np.asarray(shared_W[s]) for s in range(S)]
    cols += [np.asarray(task_W[t, k]) for t in range(T) for k in range(K)]
    wall = np.ascontiguousarray(np.concatenate(cols, axis=1), dtype=np.float16)
    bias_all = np.concatenate(
        [np.asarray(shared_b).reshape(-1), np.asarray(task_b).reshape(-1)]
    ).astype(np.float32)
    biasb = np.ascontiguousarray(np.broadcast_to(bias_all, (P, WCOLS)).astype(np.float16))
    tw1 = np.ascontiguousarray(tower_W1, dtype=np.float16)
    tb1 = np.ascontiguousarray(np.asarray(tower_b1).T, dtype=np.float32)   # [H, T]
    tw2 = np.ascontiguousarray(np.asarray(tower_W2)[:, :, 0].T, dtype=np.float16)  # [H, T]
    ident = np.eye(P, dtype=np.float16)
    return wall, biasb, tw1, tb1, tw2, ident


def kernel(
    x,
    shared_W,
    shared_b,
    task_W,
    task_b,
    gate_W,
    tower_W1,
    tower_b1,
    tower_W2,
    tower_b2,
    _trace=False,
    _tmpdir=None,
):
    nc = _get_nc()
    x = np.asarray(x, dtype=np.float32)
    wall, biasb, tw1, tb1, tw2, ident = _prep_shared(
        shared_W, shared_b, task_W, task_b, gate_W, tower_W1, tower_b1, tower_W2
    )
    in_maps = []
    for c in range(NCORES):
        xt = np.ascontiguousarray(x[c * BC: (c + 1) * BC, :].T.astype(np.float16))
        in_maps.append(
            {
                "xt": xt,
                "wall": wall,
                "biasb": biasb,
                "tw1": tw1,
                "tb1": tb1,
                "tw2": tw2,
                "ident": ident,
            }
        )
    kw = {}
    if _trace:
        kw = {"trace": True, "tmpdir": _tmpdir}
    res = run_bass_kernel_spmd(nc, in_maps, core_ids=list(range(NCORES)), **kw)
    out = np.concatenate([res.results[c]["out"] for c in range(NCORES)], axis=1)
    out = out + np.asarray(tower_b2, dtype=np.float32)[:, 0][:, None]
    result = out[:, :, None].astype(np.float32)  # [T, B, 1]
    if _trace:
        return result, res
    return result
